# revision 2
# baseline (speedup 1.0000x reference)
"""Trainium2 Bass kernel for nn_IngredientScannerLoss.

Per row (12 coords = 6 (x,y) pairs):
    delta = output - target
    dist_j = sqrt(dx_j^2 + dy_j^2)
    n_j    = (s0_j*dx_j > 0) + (s1_j*dy_j > 0)   (sign-gated count, 0/1/2)
    f(x)   = ((x+1)^1.2 - 1)*2
    t_j    = [dist, f(dist), f(f(dist))][n_j]
    loss   = sum_j t_j

Data-parallel over 8 NeuronCores: rows split 8 x 500_000, each shard
zero-padded to 501_760 = 128*560*7 rows; tiles are [128, 560*12] fp32.

v2 design notes (measured rates on HW, cyc/elem @0.96GHz):
  - subtract: fp32 TT in-place (1.02 c/e, port-bound floor). GPSIMD/Pool
    compute is avoided entirely: concurrent Pool+DVE ops serialize
    catastrophically (measured 25x stalls).
  - squares: custom DVE op (sq+sq) reading 1D stride-2 APs = 1.05 c/e;
    2D APs cost 1.71 c/e, so s stays r-major (row-major, pair fastest).
  - values (s, dist, t, W0, W1, selects) in fp16: TS 4x (0.30),
    TT 2x (0.55); fp32 delta is kept for exact strict-sign gates
    (fp16/bf16 rounding of inputs flips gates near delta=0 and single
    corrupted rows fail rel_max).
  - gate masks n/m2 stored pair-major so gate writes are contiguous;
    copy_predicated reads masks through strided APs (stride-insensitive,
    measured).
  - ACT runs 6 full-width contiguous passes (strided ACT writes cost
    4.6 c/e -- forbidden); t2/W1 computed for all 6 pairs (pairs 4,5
    results are discarded by m2=0) because a 4-pair subset would need
    strided ACT access.
  - single act table set natural_log_exp (contains ln+exp+square) via
    the get_activation_tables patch, so no per-tile table reloads.
"""

import numpy as np

import concourse.bacc as bacc
import concourse.bass as bass
import concourse.mybir as mybir
import concourse.tile as tile
from concourse import dve_ops
from concourse.bass_utils import run_bass_kernel_spmd
from concourse.dve_ops import DveOp
from concourse.dve_spec import Spec, Src0, Src1, C0, C1, Zero, _has_src1, lower, sq
from concourse.dve_uop import DveOpSpec

P = 128
COLS = 12
NPAIR = 6
B = 4_000_000
N_CORES = 8
ROWS_VALID = B // N_CORES          # 500_000
RT = 560                           # rows per partition per tile
NT = 7                             # tiles per core
ROWS_PC = P * RT * NT              # 501_760 padded rows per core
LN2 = 0.6931471805599453

# per-coordinate condition signs (see reference _SIGNS)
SIGNS = [1.0, 1.0, 1.0, -1.0, -1.0, -1.0, -1.0, 1.0, 0.0, 1.0, 0.0, -1.0]

F32 = mybir.dt.float32
F16 = mybir.dt.float16
I16 = mybir.dt.int16
AF = mybir.ActivationFunctionType
ALU = mybir.AluOpType

# ---------------------------------------------------------------- custom ops


def _register_op(name: str, spec: Spec, subdim: bool = False) -> DveOp:
    for op in dve_ops.OPS:
        if op.name == name:
            return op
    if name not in dve_ops._SUB_OPCODE_FOR_NAME:
        row = max(dve_ops._SUB_OPCODE_FOR_NAME.values()) + 1
        assert row < 0x20, "custom DVE opcode rows exhausted"
        dve_ops._SUB_OPCODE_FOR_NAME[name] = row
    shas = {}
    for ver in ("v3", "v4"):
        try:
            shas[ver] = DveOpSpec(
                name=name,
                opcode=dve_ops.get_dve_sub_opcode(name),
                uops=lower(spec, ver=ver),
                rd1_en=_has_src1(spec),
            ).sha(ver)
        except Exception:
            pass
    op = DveOp(name, spec, subdim, shas)
    dve_ops.OPS.append(op)
    dve_ops.CUSTOM_DVE_SPECS[name] = spec
    return op


# s = in0^2 + in1^2  (in0/in1 = even/odd delta columns)
PAIRDIST = _register_op(
    "ANT_PAIRDIST",
    Spec(
        body=sq(Src0) + sq(Src1),
        reference=lambda in0, in1, s0, s1, imm2: (
            in0.astype(np.float32) ** 2 + in1.astype(np.float32) ** 2
        ),
    ),
)

# n = (in0*s0 > 0) + (in1*s1 > 0)
CGATE = _register_op(
    "ANT_CGATE",
    Spec(
        body=(Src0 * C0 > Zero) + (Src1 * C1 > Zero),
        reference=lambda in0, in1, s0, s1, imm2: (
            ((in0.astype(np.float32) * s0) > 0).astype(np.float32)
            + ((in1.astype(np.float32) * s1) > 0).astype(np.float32)
        ),
    ),
)


# ---------------------------------------------------------------- act tables
# The stock table-load pass resolves Exp -> exp_and_others and
# Ln -> natural_log, reloading ACT tables on every Ln<->Exp switch
# (~2.7us each). Restrict ln/exp membership to sets that hold BOTH so
# every activation resolves to natural_log_exp_and_others and the load
# hoists to one per kernel. Dict order (act_func_set_id) is preserved.

_GAT_REAL = None


def _gat_lnexp(arch):
    global _GAT_REAL
    from concourse.hw_specs import get_activation_tables

    if _GAT_REAL is None:
        _GAT_REAL = get_activation_tables
    tabs = _GAT_REAL(arch)
    out = {}
    for name, funcs in tabs.items():
        fs = set(funcs)
        if not (AF.Ln in fs and AF.Exp in fs):
            fs.discard(AF.Ln)
            fs.discard(AF.Exp)
        out[name] = fs
    return out


def _patch_act_tables():
    if bacc.get_activation_tables is not _gat_lnexp:
        global _GAT_REAL
        _GAT_REAL = bacc.get_activation_tables
        bacc.get_activation_tables = _gat_lnexp


# ---------------------------------------------------------------- bass build


def build_nc(rt: int = RT, nt: int = NT):
    """Build the single-core SPMD program for [P*rt*nt, 12] inputs."""
    _patch_act_tables()
    rows = P * rt * nt
    w6 = rt * NPAIR          # fp16 value width (pairs)
    w12 = rt * COLS          # fp32 delta width
    nc = bacc.Bacc("TRN2", debug=False, target_bir_lowering=False,
                   num_devices=N_CORES)
    # activation biases need registered const APs (only 0.0/1.0 ship)
    for cv in (-1.0, LN2):
        if (F32, cv) not in nc.const_aps.aps:
            ct = nc.alloc_sbuf_tensor(f"const-f32-{cv}", [P, 1], F32)
            nc.gpsimd.memset(ct.ap(), cv)
            nc.const_aps.aps[(F32, cv)] = ct.ap()
    nc.all_engine_barrier()
    a = nc.dram_tensor("output", [rows, COLS], F32, kind="ExternalInput").ap()
    b = nc.dram_tensor("target", [rows, COLS], F32, kind="ExternalInput").ap()
    o = nc.dram_tensor("loss", [rows], F32, kind="ExternalOutput").ap()

    a3 = a.rearrange("(n p r) m -> n p (r m)", p=P, r=rt)
    b3 = b.rearrange("(n p r) m -> n p (r m)", p=P, r=rt)
    o3 = o.rearrange("(n p r) -> n p r", p=P, r=rt)

    with tile.TileContext(nc) as tc:
        with tc.tile_pool(name="sb", bufs=2) as pool:
            for i in range(nt):
                # ---- loads: a into delta tile (subtract runs in-place)
                delta = pool.tile([P, w12], F32, tag="delta")
                nc.sync.dma_start(out=delta[:], in_=a3[i])
                tb = pool.tile([P, w12], F32, tag="tb")
                nc.sync.dma_start(out=tb[:], in_=b3[i])

                # ---- delta = a - b (fp32, exact signs for the gates)
                nc.vector.tensor_tensor(delta[:], delta[:], tb[:],
                                        ALU.subtract)

                # ---- s = dx^2 + dy^2, r-major [p, (r,j)] fp16
                d2 = delta[:].rearrange("p (q two) -> p two q", two=2)
                slt = pool.tile([P, w6], F16, tag="slt")
                nc.vector._custom_dve(PAIRDIST, out=slt[:],
                                      in0=d2[:, 0], in1=d2[:, 1])

                # ---- gates -> n, PAIR-MAJOR [p, (j,r)] fp16 (contig writes)
                dm = delta[:].rearrange("p (r m) -> p m r", m=COLS)
                n16 = pool.tile([P, w6], F16, tag="n16")
                for j in range(NPAIR):
                    xs = slice(j * rt, (j + 1) * rt)
                    if SIGNS[2 * j] != 0.0:
                        nc.vector._custom_dve(
                            CGATE, out=n16[:, xs],
                            in0=dm[:, 2 * j], in1=dm[:, 2 * j + 1],
                            s0=SIGNS[2 * j], s1=SIGNS[2 * j + 1],
                        )
                    else:
                        op = ALU.is_gt if SIGNS[2 * j + 1] > 0 else ALU.is_lt
                        nc.vector.tensor_scalar(n16[:, xs], dm[:, 2 * j + 1],
                                                0.0, None, op)

                # ---- ACT chain, one table set (ln+exp), all contiguous:
                #   lt  = ln(s)            (in-place on slt)
                #   res = exp(0.5*lt)      = dist
                #   t   = ln(res + 1)      (in-place on slt)
                #   W0  = exp(1.2*t + ln2) = d1 + 2
                #   t2  = ln(W0 - 1)       (in-place on slt)
                #   W1  = exp(1.2*t2+ln2)  = d2 + 2  (pairs 4,5: unused)
                nc.scalar.activation(slt[:], slt[:], AF.Ln)
                res = pool.tile([P, w6], F16, tag="res")
                nc.scalar.activation(res[:], slt[:], AF.Exp, scale=0.5)
                nc.scalar.activation(slt[:], res[:], AF.Ln, bias=1.0)
                w = pool.tile([P, 2 * w6], F16, tag="w")
                nc.scalar.activation(w[:, 0:w6], slt[:], AF.Exp,
                                     scale=1.2, bias=LN2)
                nc.scalar.activation(slt[:], w[:, 0:w6], AF.Ln, bias=-1.0)
                nc.scalar.activation(w[:, w6:2 * w6], slt[:], AF.Exp,
                                     scale=1.2, bias=LN2)

                # ---- d1|d2 = W0-2 | W1-2 (one fp16 TS at 4x, in-place)
                nc.vector.tensor_scalar(w[:], w[:], 2.0, None, ALU.subtract)

                # ---- select: res overwritten by d1 where n>=1, d2 where n=2.
                # masks live pair-major; cp reads them via strided APs
                # (stride-insensitive). fp16 {0,1,2} bitcast int16 is nonzero
                # exactly where the float is.
                nmask = n16[:].rearrange("p (j r) -> p r j", j=NPAIR)
                nc.vector.copy_predicated(res[:], nmask.bitcast(I16),
                                          w[:, 0:w6])
                # m2 = relu(n-1) in place on n16 (pairs 4,5 are always 0)
                nc.vector.tensor_scalar(n16[:], n16[:], 1.0, 0.0,
                                        ALU.subtract, ALU.max)
                rv = res[:].rearrange("p (r j) -> p r j", j=NPAIR)
                dv = w[:, w6:2 * w6].rearrange("p (r j) -> p r j", j=NPAIR)
                mv = n16[:].rearrange("p (j r) -> p r j", j=NPAIR)
                nc.vector.copy_predicated(rv[:, :, 0:4],
                                          mv[:, :, 0:4].bitcast(I16),
                                          dv[:, :, 0:4])

                # ---- row sums: fp16 in-place tree over the 6 pair columns
                nc.vector.tensor_tensor(rv[:, :, 0:3], rv[:, :, 0:3],
                                        rv[:, :, 3:6], ALU.add)
                nc.vector.tensor_tensor(rv[:, :, 0], rv[:, :, 0],
                                        rv[:, :, 1], ALU.add)
                ot = pool.tile([P, rt], F32, tag="ot")
                nc.vector.tensor_tensor(ot[:], rv[:, :, 0], rv[:, :, 2],
                                        ALU.add)
                nc.sync.dma_start(out=o3[i], in_=ot[:])
    nc.compile()
    return nc


_NC_CACHE: dict = {}


def _get_nc(rt: int = RT, nt: int = NT):
    key = (rt, nt)
    if key not in _NC_CACHE:
        _NC_CACHE[key] = build_nc(rt, nt)
    return _NC_CACHE[key]


# ---------------------------------------------------------------- entrypoint


def kernel(output, target):
    a = np.asarray(output, dtype=np.float32)
    b = np.asarray(target, dtype=np.float32)
    assert a.shape == (B, COLS) and b.shape == (B, COLS)

    a_sh = np.zeros((N_CORES, ROWS_PC, COLS), dtype=np.float32)
    b_sh = np.zeros((N_CORES, ROWS_PC, COLS), dtype=np.float32)
    a_sh[:, :ROWS_VALID, :] = a.reshape(N_CORES, ROWS_VALID, COLS)
    b_sh[:, :ROWS_VALID, :] = b.reshape(N_CORES, ROWS_VALID, COLS)

    nc = _get_nc()
    in_maps = [
        {"output": a_sh[c], "target": b_sh[c]} for c in range(N_CORES)
    ]
    r = run_bass_kernel_spmd(nc, in_maps, list(range(N_CORES)))
    out = np.empty((N_CORES, ROWS_VALID), dtype=np.float32)
    for c in range(N_CORES):
        out[c] = r.results[c]["loss"][:ROWS_VALID]
    return out.reshape(B)


# revision 3
# speedup vs baseline: 1.1774x; 1.1774x over previous
"""Trainium2 Bass kernel for nn_IngredientScannerLoss.

Per row (12 coords = 6 (x,y) pairs):
    delta = output - target
    dist_j = sqrt(dx_j^2 + dy_j^2)
    n_j    = (s0_j*dx_j > 0) + (s1_j*dy_j > 0)   (sign-gated count, 0/1/2)
    f(x)   = ((x+1)^1.2 - 1)*2
    t_j    = [dist, f(dist), f(f(dist))][n_j]
    loss   = sum_j t_j

Data-parallel over 8 NeuronCores: rows split 8 x 500_000, each shard
zero-padded to 501_760 = 128*560*7 rows; tiles are [128, 560*12] fp32.

v2 design notes (measured rates on HW, cyc/elem @0.96GHz):
  - subtract: fp32 TT in-place (1.02 c/e, port-bound floor). GPSIMD/Pool
    compute is avoided entirely: concurrent Pool+DVE ops serialize
    catastrophically (measured 25x stalls).
  - squares: custom DVE op (sq+sq) reading 1D stride-2 APs = 1.05 c/e;
    2D APs cost 1.71 c/e, so s stays r-major (row-major, pair fastest).
  - values (s, dist, t, W0, W1, selects) in fp16: TS 4x (0.30),
    TT 2x (0.55); fp32 delta is kept for exact strict-sign gates
    (fp16/bf16 rounding of inputs flips gates near delta=0 and single
    corrupted rows fail rel_max).
  - gate masks n/m2 stored pair-major so gate writes are contiguous;
    copy_predicated reads masks through strided APs (stride-insensitive,
    measured).
  - ACT runs 6 full-width contiguous passes (strided ACT writes cost
    4.6 c/e -- forbidden); t2/W1 computed for all 6 pairs (pairs 4,5
    results are discarded by m2=0) because a 4-pair subset would need
    strided ACT access.
  - single act table set natural_log_exp (contains ln+exp+square) via
    the get_activation_tables patch, so no per-tile table reloads.
"""

import numpy as np

import concourse.bacc as bacc
import concourse.bass as bass
import concourse.mybir as mybir
import concourse.tile as tile
from concourse import dve_ops
from concourse.bass_utils import run_bass_kernel_spmd
from concourse.dve_ops import DveOp
from concourse.dve_spec import Spec, Src0, Src1, C0, C1, Zero, _has_src1, lower, sq
from concourse.dve_uop import DveOpSpec

P = 128
COLS = 12
NPAIR = 6
B = 4_000_000
N_CORES = 8
ROWS_VALID = B // N_CORES          # 500_000
RT = 560                           # rows per partition per tile
NT = 7                             # tiles per core
ROWS_PC = P * RT * NT              # 501_760 padded rows per core
LN2 = 0.6931471805599453

# per-coordinate condition signs (see reference _SIGNS)
SIGNS = [1.0, 1.0, 1.0, -1.0, -1.0, -1.0, -1.0, 1.0, 0.0, 1.0, 0.0, -1.0]

F32 = mybir.dt.float32
F16 = mybir.dt.float16
I16 = mybir.dt.int16
AF = mybir.ActivationFunctionType
ALU = mybir.AluOpType

# ---------------------------------------------------------------- custom ops


def _register_op(name: str, spec: Spec, subdim: bool = False) -> DveOp:
    for op in dve_ops.OPS:
        if op.name == name:
            return op
    if name not in dve_ops._SUB_OPCODE_FOR_NAME:
        row = max(dve_ops._SUB_OPCODE_FOR_NAME.values()) + 1
        assert row < 0x20, "custom DVE opcode rows exhausted"
        dve_ops._SUB_OPCODE_FOR_NAME[name] = row
    shas = {}
    for ver in ("v3", "v4"):
        try:
            shas[ver] = DveOpSpec(
                name=name,
                opcode=dve_ops.get_dve_sub_opcode(name),
                uops=lower(spec, ver=ver),
                rd1_en=_has_src1(spec),
            ).sha(ver)
        except Exception:
            pass
    op = DveOp(name, spec, subdim, shas)
    dve_ops.OPS.append(op)
    dve_ops.CUSTOM_DVE_SPECS[name] = spec
    return op


# s = in0^2 + in1^2  (in0/in1 = even/odd delta columns)
PAIRDIST = _register_op(
    "ANT_PAIRDIST",
    Spec(
        body=sq(Src0) + sq(Src1),
        reference=lambda in0, in1, s0, s1, imm2: (
            in0.astype(np.float32) ** 2 + in1.astype(np.float32) ** 2
        ),
    ),
)

# n = (in0*s0 > 0) + (in1*s1 > 0)
CGATE = _register_op(
    "ANT_CGATE",
    Spec(
        body=(Src0 * C0 > Zero) + (Src1 * C1 > Zero),
        reference=lambda in0, in1, s0, s1, imm2: (
            ((in0.astype(np.float32) * s0) > 0).astype(np.float32)
            + ((in1.astype(np.float32) * s1) > 0).astype(np.float32)
        ),
    ),
)


# ---------------------------------------------------------------- act tables
# The stock table-load pass resolves Exp -> exp_and_others and
# Ln -> natural_log, reloading ACT tables on every Ln<->Exp switch
# (~2.7us each). Restrict ln/exp membership to sets that hold BOTH so
# every activation resolves to natural_log_exp_and_others and the load
# hoists to one per kernel. Dict order (act_func_set_id) is preserved.

_GAT_REAL = None


def _gat_lnexp(arch):
    global _GAT_REAL
    from concourse.hw_specs import get_activation_tables

    if _GAT_REAL is None:
        _GAT_REAL = get_activation_tables
    tabs = _GAT_REAL(arch)
    out = {}
    for name, funcs in tabs.items():
        fs = set(funcs)
        if not (AF.Ln in fs and AF.Exp in fs):
            fs.discard(AF.Ln)
            fs.discard(AF.Exp)
        out[name] = fs
    return out


def _patch_act_tables():
    if bacc.get_activation_tables is not _gat_lnexp:
        global _GAT_REAL
        _GAT_REAL = bacc.get_activation_tables
        bacc.get_activation_tables = _gat_lnexp


# ---------------------------------------------------------------- bass build


def build_nc(rt: int = RT, nt: int = NT):
    """Build the single-core SPMD program for [P*rt*nt, 12] inputs."""
    _patch_act_tables()
    rows = P * rt * nt
    w6 = rt * NPAIR          # fp16 value width (pairs)
    w12 = rt * COLS          # fp32 delta width
    nc = bacc.Bacc("TRN2", debug=False, target_bir_lowering=False,
                   num_devices=N_CORES)
    # activation biases need registered const APs (only 0.0/1.0 ship)
    for cv in (-1.0, LN2):
        if (F32, cv) not in nc.const_aps.aps:
            ct = nc.alloc_sbuf_tensor(f"const-f32-{cv}", [P, 1], F32)
            nc.gpsimd.memset(ct.ap(), cv)
            nc.const_aps.aps[(F32, cv)] = ct.ap()
    nc.all_engine_barrier()
    a = nc.dram_tensor("output", [rows, COLS], F32, kind="ExternalInput").ap()
    b = nc.dram_tensor("target", [rows, COLS], F32, kind="ExternalInput").ap()
    o = nc.dram_tensor("loss", [rows], F32, kind="ExternalOutput").ap()

    a3 = a.rearrange("(n p r) m -> n p (r m)", p=P, r=rt)
    b3 = b.rearrange("(n p r) m -> n p (r m)", p=P, r=rt)
    o3 = o.rearrange("(n p r) -> n p r", p=P, r=rt)

    w4 = rt * 4              # pairs 0-3 prefix width (pair-major)
    with tile.TileContext(nc) as tc:
        with tc.tile_pool(name="sb", bufs=2) as pool:
            for i in range(nt):
                ta = pool.tile([P, w12], F32, tag="ta")
                nc.sync.dma_start(out=ta[:], in_=a3[i])
                tb = pool.tile([P, w12], F32, tag="tb")
                nc.sync.dma_start(out=tb[:], in_=b3[i])

                # ---- parity-split subtract (fp32, exact signs):
                #   dE = a_even - b_even -> scratch (contiguous)
                #   dO = a_odd  - b_odd  -> tb[:, 0:w6] (write lags reads:
                #        out col q at step q, reads col 2q+1 -- safe)
                av = ta[:].rearrange("p (q two) -> p two q", two=2)
                bv = tb[:].rearrange("p (q two) -> p two q", two=2)
                dE = pool.tile([P, w6], F32, tag="dE")
                nc.vector.tensor_tensor(dE[:], av[:, 0], bv[:, 0],
                                        ALU.subtract)
                nc.vector.tensor_tensor(tb[:, 0:w6], av[:, 1], bv[:, 1],
                                        ALU.subtract)
                dO = tb[:, 0:w6]

                # dE/dO element q maps to (r, j) with q = 6r + j (r-major).
                dEj = dE[:].rearrange("p (r j) -> p j r", j=NPAIR)
                dOj = dO.rearrange("p (r j) -> p j r", j=NPAIR)

                # ---- s = dx^2 + dy^2, PAIR-MAJOR [p, (j,r)] fp16
                # (j-outer 2D APs, strides 4B/24B; out contiguous)
                slt = pool.tile([P, w6], F16, tag="slt")
                nc.vector._custom_dve(PAIRDIST, out=slt[:],
                                      in0=dEj, in1=dOj)

                # ---- gates -> n, PAIR-MAJOR fp16 (contiguous writes,
                # stride-24B reads)
                n16 = pool.tile([P, w6], F16, tag="n16")
                for j in range(NPAIR):
                    xs = slice(j * rt, (j + 1) * rt)
                    if SIGNS[2 * j] != 0.0:
                        nc.vector._custom_dve(
                            CGATE, out=n16[:, xs],
                            in0=dEj[:, j], in1=dOj[:, j],
                            s0=SIGNS[2 * j], s1=SIGNS[2 * j + 1],
                        )
                    else:
                        op = ALU.is_gt if SIGNS[2 * j + 1] > 0 else ALU.is_lt
                        nc.vector.tensor_scalar(n16[:, xs], dOj[:, j],
                                                0.0, None, op)

                # ---- ACT chain, one table set (ln+exp), all contiguous,
                # pair-major; t2/W1 only on the pairs-0..3 prefix:
                #   lt  = ln(s)            (in-place on slt)
                #   res = exp(0.5*lt)      = dist
                #   t   = ln(res + 1)      (in-place on slt)
                #   W0  = exp(1.2*t + ln2) = d1 + 2
                #   t2  = ln(W0 - 1)       (in-place on slt prefix)
                #   W1  = exp(1.2*t2+ln2)  = d2 + 2
                #   m2  = relu(n - 1)      (ACT takes this; pairs 4,5 == 0)
                nc.scalar.activation(slt[:], slt[:], AF.Ln)
                res = pool.tile([P, w6], F16, tag="res")
                nc.scalar.activation(res[:], slt[:], AF.Exp, scale=0.5)
                nc.scalar.activation(slt[:], res[:], AF.Ln, bias=1.0)
                w0 = pool.tile([P, w6], F16, tag="w0")
                nc.scalar.activation(w0[:], slt[:], AF.Exp,
                                     scale=1.2, bias=LN2)
                nc.scalar.activation(slt[:, 0:w4], w0[:, 0:w4], AF.Ln,
                                     bias=-1.0)
                w1 = pool.tile([P, w4], F16, tag="w1")
                nc.scalar.activation(w1[:], slt[:, 0:w4], AF.Exp,
                                     scale=1.2, bias=LN2)

                # ---- d1 = W0-2, d2 = W1-2 (fp16 TS at 4x, in-place)
                nc.vector.tensor_scalar(w0[:], w0[:], 2.0, None, ALU.subtract)
                nc.vector.tensor_scalar(w1[:], w1[:], 2.0, None, ALU.subtract)

                # ---- select: res overwritten by d1 where n>=1, d2 where
                # n=2. All APs pair-major contiguous. fp16 {0.,1.,2.}
                # bitcast int16 is nonzero exactly where the float is.
                nc.vector.copy_predicated(res[:], n16[:].bitcast(I16), w0[:])
                # m2 = relu(n-1) on ACT (engine has headroom), prefix only
                nc.scalar.activation(n16[:, 0:w4], n16[:, 0:w4], AF.Relu,
                                     bias=-1.0)
                nc.vector.copy_predicated(res[:, 0:w4],
                                          n16[:, 0:w4].bitcast(I16), w1[:])

                # ---- row sums: in-place contiguous fp16 tree over pairs
                w3 = rt * 3
                nc.vector.tensor_tensor(res[:, 0:w3], res[:, 0:w3],
                                        res[:, w3:w6], ALU.add)
                nc.vector.tensor_tensor(res[:, 0:rt], res[:, 0:rt],
                                        res[:, rt:2 * rt], ALU.add)
                ot = pool.tile([P, rt], F32, tag="ot")
                nc.vector.tensor_tensor(ot[:], res[:, 0:rt],
                                        res[:, 2 * rt:w3], ALU.add)
                nc.sync.dma_start(out=o3[i], in_=ot[:])
    nc.compile()
    return nc


_NC_CACHE: dict = {}


def _get_nc(rt: int = RT, nt: int = NT):
    key = (rt, nt)
    if key not in _NC_CACHE:
        _NC_CACHE[key] = build_nc(rt, nt)
    return _NC_CACHE[key]


# ---------------------------------------------------------------- entrypoint


def kernel(output, target):
    a = np.asarray(output, dtype=np.float32)
    b = np.asarray(target, dtype=np.float32)
    assert a.shape == (B, COLS) and b.shape == (B, COLS)

    a_sh = np.zeros((N_CORES, ROWS_PC, COLS), dtype=np.float32)
    b_sh = np.zeros((N_CORES, ROWS_PC, COLS), dtype=np.float32)
    a_sh[:, :ROWS_VALID, :] = a.reshape(N_CORES, ROWS_VALID, COLS)
    b_sh[:, :ROWS_VALID, :] = b.reshape(N_CORES, ROWS_VALID, COLS)

    nc = _get_nc()
    in_maps = [
        {"output": a_sh[c], "target": b_sh[c]} for c in range(N_CORES)
    ]
    r = run_bass_kernel_spmd(nc, in_maps, list(range(N_CORES)))
    out = np.empty((N_CORES, ROWS_VALID), dtype=np.float32)
    for c in range(N_CORES):
        out[c] = r.results[c]["loss"][:ROWS_VALID]
    return out.reshape(B)
